# revision 32
# baseline (speedup 1.0000x reference)
"""AutoMTLSuperNet (moe_routing) Trainium2 kernel, v2.

Batch data-parallel over 8 NeuronCores (2048 samples each, params replicated).
On-chip layout is output-channel-major ([oc, batch]); all matmuls bf16 with
f32 PSUM accumulation; batch processed in chunks of 512 columns.

v2 vs v1 (352.7us -> ~328us):
- consolidated mega-DMAs: one weight pack (WB0/WB1), one input pack (xT),
  ~12 DMA issues total instead of 116 (the old prologue alone cost 54us);
- squares and gate logits share one stationary pack (SG) and one psum;
- c=0 candidate weights prescaled by softmax(alpha)[n,0] on host, so relu
  tails are plain ACT relus; c=1/2 tails are ACT + one DVE STT each;
- expert mixing: PE broadcast matmuls (selector stationaries) into PSUM,
  DVE multiplies straight from PSUM, adds on the otherwise-idle GpSimd;
- domain-select finale via a host-built onehot mask, one selector matmul,
  fast-approx reciprocal, four PE transposes and a single f32 copy out;
- engine balancing found empirically on hw traces: all relus on ACT, xq
  squares + mix-adds + z on GpSimd, everything PSUM-touching on DVE/ACT;
- PSUM tags split per layer (A=2, C=2, b1=3, sm=1 banks) to decouple
  consecutive rounds' slot reuse (big measured win: b1 at 3 banks).
Requires all-zero expert biases (true for this model).
"""

import itertools

import numpy as np
import ml_dtypes

import concourse.bass as bass
import concourse.bacc as bacc
import concourse.mybir as mybir
import concourse.tile as tile
from concourse.bass_utils import run_bass_kernel_spmd

# ---- problem dims (hardcoded per contract) ----
B, F, E, D = 16384, 26, 16, 13
NE, ND, NC = 4, 3, 3
GIN = E * (F + 1) + D            # 445
H, OUT = 256, 128
N_CORES = 8
B_LOC = B // N_CORES             # 2048
NBC = 512                        # batch columns per chunk
NCHUNK = B_LOC // NBC            # 4
KSP = F * E                      # 416 flattened sparse dim
BF16 = mybir.dt.bfloat16
F32 = mybir.dt.float32

AF = mybir.ActivationFunctionType
ALU = mybir.AluOpType

# ---- mega-pack column offsets ----
# WB0 layout (everything the small matmuls need)
OFF_SG = 0                        # 4 kt x 108  (Gs 0:64 | g0 64:80 | 0 | g1 96:108)
OFF_GQ = OFF_SG + 4 * 108         # 4 kt x 64
OFF_SEL16 = OFF_GQ + 4 * 64       # [16,4] row e*4+n -> col n
OFF_SELBR = OFF_SEL16 + 4         # [4,16] row n -> col e*4+n
OFF_S12R = OFF_SELBR + 16         # [12,36] col 0 ones; col 32+e from rows d*4+e
OFF_BC16 = OFF_S12R + 36          # 16 x [16,128] broadcast selectors
OFF_BCD = OFF_BC16 + 16 * 128     # 4 x [4,128] broadcast selectors
OFF_ID = OFF_BCD + 4 * 128        # [128,128] identity
C_WB0 = OFF_ID + 128
# WB1: [Wl0 4x3072 | Wb1 24x128 | W10 24x128 | W11 24x128]
OFF_WL0 = 0
OFF_WB1 = OFF_WL0 + 4 * 3072
OFF_W10 = OFF_WB1 + 24 * 128
OFF_W11 = OFF_W10 + 24 * 128
C_WB1 = OFF_W11 + 24 * 128


def _bf16(x):
    return np.asarray(x, dtype=ml_dtypes.bfloat16)


def _softmax_np(a):
    a = np.asarray(a, dtype=np.float64)
    m = a.max(axis=-1, keepdims=True)
    e = np.exp(a - m)
    return (e / e.sum(axis=-1, keepdims=True)).astype(np.float32)


# ============================================================================
# host prep
# ============================================================================

def prep_shared_v2(inputs):
    f32 = np.float32
    gate_w = 1.0 / (1.0 + np.exp(-inputs['feat_alpha'].astype(np.float64)))
    gate_w = gate_w.astype(f32)                                # [NE,F]

    W_l0b0 = inputs['W_l0b0'].astype(f32)   # [NE,NC,GIN,H]
    W_l0b1 = inputs['W_l0b1'].astype(f32)   # [NE,NC,H,OUT]
    W_l1b0 = inputs['W_l1b0'].astype(f32)   # [NE,NC,OUT,H]
    W_l1b1 = inputs['W_l1b1'].astype(f32)   # [NE,NC,H,OUT]

    wmix = [_softmax_np(inputs[k]) for k in ('a_l0b0', 'a_l0b1', 'a_l1b0', 'a_l1b1')]

    # ---- WB0 ----
    WB0 = np.zeros((128, C_WB0), dtype=f32)
    Wg0, Wg1 = inputs['Wg0'].astype(f32), inputs['Wg1'].astype(f32)
    for fe in range(KSP):
        kt, i = divmod(fe, 128)
        f_, e_ = divmod(fe, E)
        col0 = OFF_SG + kt * 108
        for n in range(NE):
            WB0[i, col0 + n * 16 + e_] = gate_w[n, f_]
        for n in range(NE):
            for e in range(NE):
                WB0[i, col0 + 64 + e * 4 + n] = Wg0[n, fe, e]
        for d in range(ND):
            for e in range(NE):
                WB0[i, col0 + 96 + d * 4 + e] = Wg1[d, fe, e]
    for fe in range(KSP):
        kt, i = divmod(fe, 128)
        f_, e_ = divmod(fe, E)
        for n in range(NE):
            g = gate_w[n, f_]
            WB0[i, OFF_GQ + kt * 64 + n * 16 + e_] = 0.5 * g * g
    for e in range(NE):
        for n in range(NE):
            WB0[e * 4 + n, OFF_SEL16 + n] = 1.0
    for n in range(NE):
        for e in range(NE):
            WB0[n, OFF_SELBR + e * 4 + n] = 1.0
    # col 0 = ones (rsum -> partition 0, where recip_approx_fast is safe);
    # cols 32+e = domain-select rows (partition-32-aligned for the TT)
    WB0[0:12, OFF_S12R] = 1.0
    for d in range(ND):
        for e in range(NE):
            WB0[d * 4 + e, OFF_S12R + 32 + e] = 1.0
    for k in range(16):
        WB0[k, OFF_BC16 + k * 128: OFF_BC16 + (k + 1) * 128] = 1.0
    for e in range(NE):
        WB0[e, OFF_BCD + e * 128: OFF_BCD + (e + 1) * 128] = 1.0
    WB0[:, OFF_ID:OFF_ID + 128] = np.eye(128, dtype=f32)

    # ---- WB1 ----
    WB1 = np.zeros((128, C_WB1), dtype=f32)
    # Wl0 kt-tiles [128, 3072], col within kt = (n*6 + c*2 + hh)*128 + h'
    Wsp = np.zeros((KSP, NE, NC, H), dtype=f32)
    for n in range(NE):
        gvec = np.repeat(gate_w[n], E)
        Wsp[:, n] = W_l0b0[n, :, :KSP, :].transpose(1, 0, 2) * gvec[:, None, None]
    for n in range(NE):
        Wsp[:, n, 0, :] *= wmix[0][n, 0]
    Wsp = Wsp.reshape(KSP, NE * NC * H)
    for kt in range(3):
        WB1[:, OFF_WL0 + kt * 3072: OFF_WL0 + (kt + 1) * 3072] = Wsp[kt * 128:(kt + 1) * 128]
    col3 = OFF_WL0 + 3 * 3072
    WB1[0:32, col3: col3 + 3072] = Wsp[384:416]
    w0sc = np.ones((NE, NC), dtype=f32)
    w0sc[:, 0] = wmix[0][:, 0]
    for d in range(D):
        v = W_l0b0[:, :, KSP + E + d, :] * w0sc[:, :, None]        # [NE,NC,H]
        WB1[32 + d, col3: col3 + 3072] = v.reshape(-1)
    for n in range(NE):
        for e in range(E):
            v = W_l0b0[n, :, KSP + e, :] * w0sc[n, :, None]        # [NC,H]
            WB1[64 + n * 16 + e, col3 + n * 768: col3 + (n + 1) * 768] = v.reshape(768)
    for n in range(NE):
        for c in range(NC):
            w = W_l0b1[n, c] * (wmix[1][n, 0] if c == 0 else 1.0)  # [H, OUT]
            for hh in range(2):
                cc0 = OFF_WB1 + ((n * 2 + hh) * 3 + c) * 128
                WB1[:, cc0:cc0 + 128] = w[hh * 128:(hh + 1) * 128, :]
    for n in range(NE):
        for c in range(NC):
            w = W_l1b0[n, c] * (wmix[2][n, 0] if c == 0 else 1.0)  # [OUT, H]
            for hh in range(2):
                cc0 = OFF_W10 + (n * 6 + c * 2 + hh) * 128
                WB1[:, cc0:cc0 + 128] = w[:, hh * 128:(hh + 1) * 128]
    for n in range(NE):
        for c in range(NC):
            w = W_l1b1[n, c] * (wmix[3][n, 0] if c == 0 else 1.0)  # [H, OUT]
            for hh in range(2):
                cc0 = OFF_W11 + ((n * 2 + hh) * 3 + c) * 128
                WB1[:, cc0:cc0 + 128] = w[hh * 128:(hh + 1) * 128, :]

    # ---- exp bias [44,1]: rows 0:16 g0 (e*4+n), rows 32:44 g1 (d*4+e) ----
    gbias = np.zeros((44, 1), dtype=f32)
    for n in range(NE):
        for e in range(NE):
            gbias[e * 4 + n, 0] = inputs['bg0'][n, e] + inputs['beta0'][n, e]
    for d in range(ND):
        for e in range(NE):
            gbias[32 + d * 4 + e, 0] = inputs['bg1'][d, e] + inputs['beta1'][d, e]

    return {'WB0': _bf16(WB0), 'WB1': _bf16(WB1), 'gbias': gbias}, wmix


def prep_core_v2(inputs, r):
    lo, hi = r * B_LOC, (r + 1) * B_LOC
    xs = inputs['sparse_embs'][lo:hi].reshape(B_LOC, KSP).astype(np.float32)
    xT = np.zeros((128, 4 * B_LOC), dtype=ml_dtypes.bfloat16)
    xsT = _bf16(xs.T)                                     # [416, B_LOC]
    for kt in range(3):
        xT[:, kt * B_LOC:(kt + 1) * B_LOC] = xsT[kt * 128:(kt + 1) * 128]
    xT[0:32, 3 * B_LOC:4 * B_LOC] = xsT[384:416]
    xT[32:45, 3 * B_LOC:4 * B_LOC] = _bf16(inputs['dense_features'][lo:hi].astype(np.float32).T)
    # onehot12 rows d*4+e = [domain==d]
    dom = inputs['domain_ids'][lo:hi].astype(np.int64)
    oh12 = np.zeros((12, B_LOC), dtype=ml_dtypes.bfloat16)
    for d in range(ND):
        m = (dom == d).astype(np.float32)
        for e in range(NE):
            oh12[d * 4 + e] = _bf16(m)
    return {'xT': xT, 'oh12': oh12}


# ============================================================================
# program
# ============================================================================

def build_program_v2(wmix):
    nc = bacc.Bacc(trn_type="TRN2", target_bir_lowering=False, debug=False)

    t_xT = nc.dram_tensor('xT', [128, 4 * B_LOC], BF16, kind="ExternalInput").ap()
    t_oh = nc.dram_tensor('oh12', [12, B_LOC], BF16, kind="ExternalInput").ap()
    t_WB0 = nc.dram_tensor('WB0', [128, C_WB0], BF16, kind="ExternalInput").ap()
    t_WB1 = nc.dram_tensor('WB1', [128, C_WB1], BF16, kind="ExternalInput").ap()
    t_gb = nc.dram_tensor('gbias', [44, 1], F32, kind="ExternalInput").ap()
    t_out = nc.dram_tensor('out', [B_LOC, OUT], F32, kind="ExternalOutput").ap()

    uid = itertools.count()

    with tile.TileContext(nc) as tc:
        with (
            tc.tile_pool(name="wpool", bufs=1) as wpool,
            tc.tile_pool(name="xqpool", bufs=3) as xqpool,
            tc.tile_pool(name="gpool", bufs=2) as gpool,
            tc.tile_pool(name="apool", bufs=3) as apool,
            tc.tile_pool(name="hpool", bufs=2) as hpool,
            tc.tile_pool(name="bcpool", bufs=5) as bcpool,
            tc.tile_pool(name="opool", bufs=2) as opool,
            tc.tile_pool(name="ps_big", bufs=2, space="PSUM") as ps_big,
            tc.tile_pool(name="ps_b1", bufs=3, space="PSUM") as ps_b1,
            tc.tile_pool(name="ps_sm", bufs=1, space="PSUM") as ps_sm,
        ):
            # ---- prologue DMAs ----
            sWB0 = wpool.tile([128, C_WB0], BF16, tag="WB0", name="WB0")
            nc.sync.dma_start(sWB0[:], t_WB0)
            sGb = wpool.tile([44, 1], F32, tag="gbias", name="gbias")
            nc.sync.dma_start(sGb[:], t_gb)
            sXT = wpool.tile([128, 4 * B_LOC], BF16, tag="xT", name="xT")
            xr_d = t_xT.rearrange("p (k b) -> p k b", k=4)
            xr_s = sXT[:].rearrange("p (k b) -> p k b", k=4)
            nc.sync.dma_start(xr_s[:, :, 0:NBC], xr_d[:, :, 0:NBC])
            nc.sync.dma_start(xr_s[:, :, NBC:B_LOC], xr_d[:, :, NBC:B_LOC])
            # oh12 lands at partitions 32:44 so the phase4 TT against
            # e1 (gexp[32:44]) has matching base partitions
            sOh = wpool.tile([44, B_LOC], BF16, tag="oh12", name="oh12")
            nc.sync.dma_start(sOh[32:44, :], t_oh)
            sWB1 = wpool.tile([128, C_WB1], BF16, tag="WB1", name="WB1")
            nc.sync.dma_start(sWB1[:, OFF_WL0:OFF_WB1], t_WB1[:, OFF_WL0:OFF_WB1])
            nc.sync.dma_start(sWB1[:, OFF_WB1:C_WB1], t_WB1[:, OFF_WB1:C_WB1])

            def sg_l(kt):
                rows = 128 if kt < 3 else 64
                return sWB0[0:rows, OFF_SG + kt * 108: OFF_SG + (kt + 1) * 108]

            def gq_l(kt):
                rows = 128 if kt < 3 else 64
                return sWB0[0:rows, OFF_GQ + kt * 64: OFF_GQ + (kt + 1) * 64]

            sSel16 = sWB0[0:16, OFF_SEL16:OFF_SEL16 + 4]
            sSelBr = sWB0[0:4, OFF_SELBR:OFF_SELBR + 16]
            sS12R = sWB0[0:12, OFF_S12R:OFF_S12R + 36]
            sId = sWB0[:, OFF_ID:OFF_ID + 128]

            def bc16(k):
                return sWB0[0:16, OFF_BC16 + k * 128: OFF_BC16 + (k + 1) * 128]

            def bcd(e):
                return sWB0[0:4, OFF_BCD + e * 128: OFF_BCD + (e + 1) * 128]

            def wl0(kt, m):
                return sWB1[:, OFF_WL0 + kt * 3072 + m * 128: OFF_WL0 + kt * 3072 + (m + 1) * 128]

            def wb1(n, hh, c):
                o = OFF_WB1 + ((n * 2 + hh) * 3 + c) * 128
                return sWB1[:, o:o + 128]

            def w10(n, c, hh):
                o = OFF_W10 + (n * 6 + c * 2 + hh) * 128
                return sWB1[:, o:o + 128]

            def w11(n, hh, c):
                o = OFF_W11 + ((n * 2 + hh) * 3 + c) * 128
                return sWB1[:, o:o + 128]

            def xblk(kt, cc, rows=128):
                return sXT[0:rows, kt * B_LOC + cc: kt * B_LOC + cc + NBC]

            # per-chunk state
            e0n = [None] * NCHUNK
            e1bf = [None] * NCHUNK
            hAd = [None] * NCHUNK
            hB = [None] * NCHUNK
            mixed = [None] * NCHUNK
            hCd = [None] * NCHUNK
            h2 = [None] * NCHUNK

            # ============ P0: squares, gates, fm, softmax prep ============
            def phase0(ch):
                cc = ch * NBC
                sg_ps = ps_sm.tile([108, NBC], F32, tag="sm", name=f"sg_{ch}")
                for kt in range(4):
                    rows = 128 if kt < 3 else 64
                    nc.tensor.matmul(sg_ps[:], sg_l(kt), xblk(kt, cc, rows),
                                     start=(kt == 0), stop=(kt == 3))
                xq = xqpool.tile([128, 4 * NBC], BF16, tag="xq", name=f"xq_{ch}")
                for kt in range(3):
                    nc.vector.tensor_tensor(xq[:, kt * NBC:(kt + 1) * NBC],
                                            xblk(kt, cc), xblk(kt, cc), ALU.mult)
                nc.vector.tensor_tensor(xq[0:64, 3 * NBC:4 * NBC],
                                        xblk(3, cc, 64), xblk(3, cc, 64), ALU.mult)
                q_ps = ps_c.tile([64, NBC], F32, tag="c", name=f"q_{ch}")
                for kt in range(4):
                    rows = 128 if kt < 3 else 64
                    nc.tensor.matmul(q_ps[:], gq_l(kt),
                                     xq[0:rows, kt * NBC: kt * NBC + NBC],
                                     start=(kt == 0), stop=(kt == 3))
                ssq = gpool.tile([64, NBC], F32, tag="ssq", name=f"ssq_{ch}")
                nc.scalar.activation(ssq[:], sg_ps[0:64, :], AF.Square,
                                     scale=float(np.sqrt(0.5)))
                # gate exp (bias folded) -> bf16; rows 0:16 = e0, 32:44 = e1
                gexp = gpool.tile([44, NBC], BF16, tag="gexp", name=f"gexp_{ch}",
                                  bufs=NCHUNK)
                nc.scalar.activation(gexp[:], sg_ps[64:108, :], AF.Exp, bias=sGb[:, 0:1])
                e1bf[ch] = gexp[32:44, :]
                # fm rows -> xT kt3 block partitions 64:128
                nc.vector.tensor_tensor(sXT[64:128, 3 * B_LOC + cc: 3 * B_LOC + cc + NBC],
                                        ssq[:], q_ps[:], ALU.subtract)
                # expert-gate softmax: e0n = e0 * bcast16(1/sum)
                s_ps = ps_sm.tile([4, NBC], F32, tag="sm", name=f"s_{ch}")
                nc.tensor.matmul(s_ps[:], sSel16, gexp[0:16, :], start=True, stop=True)
                r0f = gpool.tile([4, NBC], F32, tag="r0f", name=f"r0f_{ch}")
                nc.vector.reciprocal_approx_fast(r0f[:], s_ps[:])
                r0 = gpool.tile([4, NBC], BF16, tag="r0", name=f"r0_{ch}")
                nc.scalar.copy(r0[:], r0f[:])
                rbc = ps_sm.tile([16, NBC], F32, tag="sm", name=f"rbc_{ch}")
                nc.tensor.matmul(rbc[:], sSelBr, r0[:], start=True, stop=True)
                t = gpool.tile([16, NBC], BF16, tag="e0n", name=f"e0n_{ch}",
                               bufs=NCHUNK)
                nc.vector.tensor_tensor(t[:], gexp[0:16, :], rbc[:], ALU.mult)
                e0n[ch] = t

            # ============ P1 pieces ============
            def l0b0_n(ch, n):
                cc = ch * NBC
                ht = hpool.tile([128, 2 * NBC], BF16, tag=f"hA{n}", name=f"hA{n}_{ch}")
                hAd[ch][n] = ht
                for c in range(NC):
                    p = ps_big.tile([128, 2 * NBC], F32, tag="big",
                                    name=f"pA{n}{c}_{ch}")
                    for hh in range(2):
                        m = n * 6 + c * 2 + hh
                        for kt in range(4):
                            nc.tensor.matmul(p[:, hh * NBC:(hh + 1) * NBC],
                                             wl0(kt, m), xblk(kt, cc),
                                             start=(kt == 0), stop=(kt == 3))
                    if c == 0:
                        nc.scalar.activation(ht[:], p[:], AF.Relu)
                    else:
                        fn = AF.Gelu_apprx_tanh if c == 1 else AF.Tanh
                        tmp = apool.tile([128, 2 * NBC], BF16, tag=f"td{c}",
                                         name=f"td{c}_{next(uid)}")
                        nc.scalar.activation(tmp[:], p[:], fn)
                        nc.vector.scalar_tensor_tensor(ht[:], tmp[:],
                                                       float(wmix[0][n, c]), ht[:],
                                                       ALU.mult, ALU.add)

            def l0b1_n(ch, n):
                ht = hpool.tile([128, NBC], BF16, tag=f"hB{n}", name=f"hB{n}_{ch}")
                hB[ch][n] = ht
                for c in range(NC):
                    p = ps_b1.tile([128, NBC], F32, tag="b1", name=f"pB{n}{c}_{ch}")
                    for hh in range(2):
                        nc.tensor.matmul(p[:], wb1(n, hh, c),
                                         hAd[ch][n][:, hh * NBC:(hh + 1) * NBC],
                                         start=(hh == 0), stop=(hh == 1))
                    if c == 0:
                        nc.scalar.activation(ht[:], p[:], AF.Relu)
                    else:
                        fn = AF.Gelu_apprx_tanh if c == 1 else AF.Tanh
                        tmp = apool.tile([128, NBC], BF16, tag=f"ts{c}",
                                         name=f"ts{c}_{next(uid)}")
                        nc.scalar.activation(tmp[:], p[:], fn)
                        nc.vector.scalar_tensor_tensor(ht[:], tmp[:],
                                                       float(wmix[1][n, c]), ht[:],
                                                       ALU.mult, ALU.add)

            # ============ P2: expert mixing for one n ============
            # multiplies on DVE (GpSimd cannot read PSUM), adds on Pool
            def phase2_n(ch, n):
                mul_eng = nc.vector
                add_eng = nc.gpsimd
                acc = bcpool.tile([128, NBC], BF16, tag=f"mix{n}", name=f"mix{n}_{ch}")
                mixed[ch][n] = acc
                for e in range(NE):
                    bp = ps_b1.tile([128, NBC], F32, tag="bp", name=f"bp{n}{e}_{ch}")
                    nc.tensor.matmul(bp[:], bc16(e * 4 + n), e0n[ch][:],
                                     start=True, stop=True)
                    if e == 0:
                        mul_eng.tensor_tensor(acc[:], hB[ch][0][:], bp[:], ALU.mult)
                    else:
                        t2 = bcpool.tile([128, NBC], BF16, tag="mixt",
                                         name=f"mixt{n}{e}_{ch}")
                        mul_eng.tensor_tensor(t2[:], hB[ch][e][:], bp[:], ALU.mult)
                        add_eng.tensor_tensor(acc[:], acc[:], t2[:], ALU.add)

            # ============ P3: L1b0 -> hCd ; L1b1 -> h2 ============
            def l1b0_n(ch, n):
                ht = hpool.tile([128, 2 * NBC], BF16, tag=f"hC{n}", name=f"hC{n}_{ch}")
                hCd[ch][n] = ht
                for c in range(NC):
                    p = ps_big.tile([128, 2 * NBC], F32, tag="big",
                                    name=f"pC{n}{c}_{ch}")
                    for hh in range(2):
                        nc.tensor.matmul(p[:, hh * NBC:(hh + 1) * NBC],
                                         w10(n, c, hh), mixed[ch][n][:],
                                         start=True, stop=True)
                    if c == 0:
                        nc.scalar.activation(ht[:], p[:], AF.Relu)
                    else:
                        fn = AF.Gelu_apprx_tanh if c == 1 else AF.Tanh
                        tmp = apool.tile([128, 2 * NBC], BF16, tag=f"td{c}",
                                         name=f"tc{c}_{next(uid)}")
                        nc.scalar.activation(tmp[:], p[:], fn)
                        nc.vector.scalar_tensor_tensor(ht[:], tmp[:],
                                                       float(wmix[2][n, c]), ht[:],
                                                       ALU.mult, ALU.add)

            def l1b1_n(ch, n):
                ht = hpool.tile([128, NBC], BF16, tag=f"h2{n}", name=f"h2{n}_{ch}")
                h2[ch][n] = ht
                for c in range(NC):
                    p = ps_b1.tile([128, NBC], F32, tag="b1", name=f"pD{n}{c}_{ch}")
                    for hh in range(2):
                        nc.tensor.matmul(p[:], w11(n, hh, c),
                                         hCd[ch][n][:, hh * NBC:(hh + 1) * NBC],
                                         start=(hh == 0), stop=(hh == 1))
                    if c == 0:
                        nc.vector.tensor_scalar(ht[:], p[:], 0.0, None, ALU.max)
                    else:
                        fn = AF.Gelu_apprx_tanh if c == 1 else AF.Tanh
                        tmp = apool.tile([128, NBC], BF16, tag=f"ts{c}",
                                         name=f"tu{c}_{next(uid)}")
                        nc.scalar.activation(tmp[:], p[:], fn)
                        nc.vector.scalar_tensor_tensor(ht[:], tmp[:],
                                                       float(wmix[3][n, c]), ht[:],
                                                       ALU.mult, ALU.add)

            # ============ P4: domain softmax-select + output ============
            def phase4(ch):
                cc = ch * NBC
                z = opool.tile([12, NBC], BF16, tag="z", name=f"z_{ch}", bufs=1)
                nc.gpsimd.tensor_tensor(z[:], e1bf[ch], sOh[32:44, cc:cc + NBC], ALU.mult)
                rs_ps = ps_sm.tile([36, NBC], F32, tag="sm", name=f"rs_{ch}")
                nc.tensor.matmul(rs_ps[:], sS12R, z[:], start=True, stop=True)
                rinv = opool.tile([1, NBC], F32, tag="rinv", name=f"rinv_{ch}", bufs=1)
                nc.vector.reciprocal_approx_fast(rinv[:], rs_ps[0:1, :])
                rb4 = opool.tile([4, NBC], F32, tag="rb4", name=f"rb4_{ch}", bufs=1)
                nc.gpsimd.partition_broadcast(rb4[:], rinv[:])
                dseln = opool.tile([4, NBC], BF16, tag="dseln", name=f"dseln_{ch}", bufs=1)
                nc.vector.tensor_tensor(dseln[:], rs_ps[32:36, :], rb4[:], ALU.mult)
                oacc = opool.tile([128, NBC], BF16, tag="oacc", name=f"oacc_{ch}", bufs=1)
                for e in range(NE):
                    bp = ps_b1.tile([128, NBC], F32, tag="bp", name=f"bpd{e}_{ch}")
                    nc.tensor.matmul(bp[:], bcd(e), dseln[:], start=True, stop=True)
                    if e == 0:
                        nc.vector.tensor_tensor(oacc[:], h2[ch][0][:], bp[:], ALU.mult)
                    else:
                        t2 = opool.tile([128, NBC], BF16, tag="omixt",
                                        name=f"ot{e}_{ch}", bufs=1)
                        nc.vector.tensor_tensor(t2[:], h2[ch][e][:], bp[:], ALU.mult)
                        nc.gpsimd.tensor_tensor(oacc[:], oacc[:], t2[:], ALU.add)
                tp = ps_sm.tile([128, NBC], BF16, tag="sm", name=f"tp_{ch}")
                for bt in range(4):
                    nc.tensor.transpose(tp[:, bt * 128:(bt + 1) * 128],
                                        oacc[:, bt * 128:(bt + 1) * 128], sId)
                oF = opool.tile([128, NBC], F32, tag="oF", name=f"oF_{ch}")
                nc.scalar.copy(oF[:], tp[:])
                for bt in range(4):
                    nc.sync.dma_start(t_out[cc + bt * 128: cc + (bt + 1) * 128, :],
                                      oF[:, bt * 128:(bt + 1) * 128])

            # ---- emission schedule: ph2(ch-1) interleaves into ph1(ch);
            # phase0 of later chunks spreads into earlier rounds; the
            # epilogue (last chunk's back half) chains per-n so it pipelines.
            phase0(0)
            phase0(1)
            for ch in range(NCHUNK):
                hAd[ch] = {}
                hB[ch] = {}
                if ch > 0:
                    mixed[ch - 1] = {}
                for n in range(NE):
                    l0b0_n(ch, n)
                    if ch > 0:
                        phase2_n(ch - 1, n)
                    if n == 1 and ch + 2 < NCHUNK:
                        phase0(ch + 2)
                for n in range(NE):
                    l0b1_n(ch, n)
                if ch > 0:
                    hCd[ch - 1] = {}
                    h2[ch - 1] = {}
                    for n in range(NE):
                        l1b0_n(ch - 1, n)
                    for n in range(NE):
                        l1b1_n(ch - 1, n)
                    phase4(ch - 1)
            ch = NCHUNK - 1
            mixed[ch] = {}
            hCd[ch] = {}
            h2[ch] = {}
            for n in range(NE):
                phase2_n(ch, n)
                l1b0_n(ch, n)
                l1b1_n(ch, n)
            phase4(ch)
    nc.compile()
    return nc


_CACHE = {}


def kernel(**inputs):
    zero_bias = all(np.abs(inputs[k]).max() == 0.0
                    for k in ('b_l0b0', 'b_l0b1', 'b_l1b0', 'b_l1b1'))
    if not zero_bias:
        raise NotImplementedError("v2 kernel requires zero expert biases")
    shared, wmix = prep_shared_v2(inputs)
    in_maps = []
    for r in range(N_CORES):
        m = dict(shared)
        m.update(prep_core_v2(inputs, r))
        in_maps.append(m)
    key = tuple(np.concatenate([w.ravel() for w in wmix]).tolist())
    if _CACHE.get('key') != key:
        _CACHE['nc'] = build_program_v2(wmix)
        _CACHE['key'] = key
    nc = _CACHE['nc']
    res = run_bass_kernel_spmd(nc, in_maps, core_ids=list(range(N_CORES)))
    out = np.concatenate([res.results[r]['out'] for r in range(N_CORES)], axis=0)
    return out.astype(np.float32)


# revision 33
# speedup vs baseline: 1.1826x; 1.1826x over previous
"""AutoMTLSuperNet (moe_routing) Trainium2 kernel, v2.

Batch data-parallel over 8 NeuronCores (2048 samples each, params replicated).
On-chip layout is output-channel-major ([oc, batch]); all matmuls bf16 with
f32 PSUM accumulation; batch processed in chunks of 512 columns.

v2 vs v1 (352.7us -> ~328us):
- consolidated mega-DMAs: one weight pack (WB0/WB1), one input pack (xT),
  ~12 DMA issues total instead of 116 (the old prologue alone cost 54us);
- squares and gate logits share one stationary pack (SG) and one psum;
- c=0 candidate weights prescaled by softmax(alpha)[n,0] on host, so relu
  tails are plain ACT relus; c=1/2 tails are ACT + one DVE STT each;
- expert mixing: PE broadcast matmuls (selector stationaries) into PSUM,
  DVE multiplies straight from PSUM, adds on the otherwise-idle GpSimd;
- domain-select finale via a host-built onehot mask, one selector matmul,
  fast-approx reciprocal, four PE transposes and a single f32 copy out;
- engine balancing found empirically on hw traces: all relus on ACT, xq
  squares + mix-adds + z on GpSimd, everything PSUM-touching on DVE/ACT;
- PSUM tags split per layer (A=2, C=2, b1=3, sm=1 banks) to decouple
  consecutive rounds' slot reuse (big measured win: b1 at 3 banks).
Requires all-zero expert biases (true for this model).
"""

import itertools

import numpy as np
import ml_dtypes

import concourse.bass as bass
import concourse.bacc as bacc
import concourse.mybir as mybir
import concourse.tile as tile
from concourse.bass_utils import run_bass_kernel_spmd

# ---- problem dims (hardcoded per contract) ----
B, F, E, D = 16384, 26, 16, 13
NE, ND, NC = 4, 3, 3
GIN = E * (F + 1) + D            # 445
H, OUT = 256, 128
N_CORES = 8
B_LOC = B // N_CORES             # 2048
NBC = 512                        # batch columns per chunk
NCHUNK = B_LOC // NBC            # 4
KSP = F * E                      # 416 flattened sparse dim
BF16 = mybir.dt.bfloat16
F32 = mybir.dt.float32

AF = mybir.ActivationFunctionType
ALU = mybir.AluOpType

# ---- mega-pack column offsets ----
# WB0 layout (everything the small matmuls need)
OFF_SG = 0                        # 4 kt x 108  (Gs 0:64 | g0 64:80 | 0 | g1 96:108)
OFF_GQ = OFF_SG + 4 * 108         # 4 kt x 64
OFF_SEL16 = OFF_GQ + 4 * 64       # [16,4] row e*4+n -> col n
OFF_SELBR = OFF_SEL16 + 4         # [4,16] row n -> col e*4+n
OFF_S12R = OFF_SELBR + 16         # [12,36] col 0 ones; col 32+e from rows d*4+e
OFF_BC16 = OFF_S12R + 36          # 16 x [16,128] broadcast selectors
OFF_BCD = OFF_BC16 + 16 * 128     # 4 x [4,128] broadcast selectors
OFF_ID = OFF_BCD + 4 * 128        # [128,128] identity
C_WB0 = OFF_ID + 128
# WB1: [Wl0 4x3072 | Wb1 24x128 | W10 24x128 | W11 24x128]
OFF_WL0 = 0
OFF_WB1 = OFF_WL0 + 4 * 3072
OFF_W10 = OFF_WB1 + 24 * 128
OFF_W11 = OFF_W10 + 24 * 128
C_WB1 = OFF_W11 + 24 * 128


def _bf16(x):
    return np.asarray(x, dtype=ml_dtypes.bfloat16)


def _softmax_np(a):
    a = np.asarray(a, dtype=np.float64)
    m = a.max(axis=-1, keepdims=True)
    e = np.exp(a - m)
    return (e / e.sum(axis=-1, keepdims=True)).astype(np.float32)


# ============================================================================
# host prep
# ============================================================================

def prep_shared_v2(inputs):
    f32 = np.float32
    gate_w = 1.0 / (1.0 + np.exp(-inputs['feat_alpha'].astype(np.float64)))
    gate_w = gate_w.astype(f32)                                # [NE,F]

    W_l0b0 = inputs['W_l0b0'].astype(f32)   # [NE,NC,GIN,H]
    W_l0b1 = inputs['W_l0b1'].astype(f32)   # [NE,NC,H,OUT]
    W_l1b0 = inputs['W_l1b0'].astype(f32)   # [NE,NC,OUT,H]
    W_l1b1 = inputs['W_l1b1'].astype(f32)   # [NE,NC,H,OUT]

    wmix = [_softmax_np(inputs[k]) for k in ('a_l0b0', 'a_l0b1', 'a_l1b0', 'a_l1b1')]

    # ---- WB0 ----
    WB0 = np.zeros((128, C_WB0), dtype=f32)
    Wg0, Wg1 = inputs['Wg0'].astype(f32), inputs['Wg1'].astype(f32)
    for fe in range(KSP):
        kt, i = divmod(fe, 128)
        f_, e_ = divmod(fe, E)
        col0 = OFF_SG + kt * 108
        for n in range(NE):
            WB0[i, col0 + n * 16 + e_] = gate_w[n, f_]
        for n in range(NE):
            for e in range(NE):
                WB0[i, col0 + 64 + e * 4 + n] = Wg0[n, fe, e]
        for d in range(ND):
            for e in range(NE):
                WB0[i, col0 + 96 + d * 4 + e] = Wg1[d, fe, e]
    for fe in range(KSP):
        kt, i = divmod(fe, 128)
        f_, e_ = divmod(fe, E)
        for n in range(NE):
            g = gate_w[n, f_]
            WB0[i, OFF_GQ + kt * 64 + n * 16 + e_] = 0.5 * g * g
    for e in range(NE):
        for n in range(NE):
            WB0[e * 4 + n, OFF_SEL16 + n] = 1.0
    for n in range(NE):
        for e in range(NE):
            WB0[n, OFF_SELBR + e * 4 + n] = 1.0
    # col 0 = ones (rsum -> partition 0, where recip_approx_fast is safe);
    # cols 32+e = domain-select rows (partition-32-aligned for the TT)
    WB0[0:12, OFF_S12R] = 1.0
    for d in range(ND):
        for e in range(NE):
            WB0[d * 4 + e, OFF_S12R + 32 + e] = 1.0
    for k in range(16):
        WB0[k, OFF_BC16 + k * 128: OFF_BC16 + (k + 1) * 128] = 1.0
    for e in range(NE):
        WB0[e, OFF_BCD + e * 128: OFF_BCD + (e + 1) * 128] = 1.0
    WB0[:, OFF_ID:OFF_ID + 128] = np.eye(128, dtype=f32)

    # ---- WB1 ----
    WB1 = np.zeros((128, C_WB1), dtype=f32)
    # Wl0 kt-tiles [128, 3072], col within kt = (n*6 + c*2 + hh)*128 + h'
    Wsp = np.zeros((KSP, NE, NC, H), dtype=f32)
    for n in range(NE):
        gvec = np.repeat(gate_w[n], E)
        Wsp[:, n] = W_l0b0[n, :, :KSP, :].transpose(1, 0, 2) * gvec[:, None, None]
    for n in range(NE):
        Wsp[:, n, 0, :] *= wmix[0][n, 0]
    Wsp = Wsp.reshape(KSP, NE * NC * H)
    for kt in range(3):
        WB1[:, OFF_WL0 + kt * 3072: OFF_WL0 + (kt + 1) * 3072] = Wsp[kt * 128:(kt + 1) * 128]
    col3 = OFF_WL0 + 3 * 3072
    WB1[0:32, col3: col3 + 3072] = Wsp[384:416]
    w0sc = np.ones((NE, NC), dtype=f32)
    w0sc[:, 0] = wmix[0][:, 0]
    for d in range(D):
        v = W_l0b0[:, :, KSP + E + d, :] * w0sc[:, :, None]        # [NE,NC,H]
        WB1[32 + d, col3: col3 + 3072] = v.reshape(-1)
    for n in range(NE):
        for e in range(E):
            v = W_l0b0[n, :, KSP + e, :] * w0sc[n, :, None]        # [NC,H]
            WB1[64 + n * 16 + e, col3 + n * 768: col3 + (n + 1) * 768] = v.reshape(768)
    for n in range(NE):
        for c in range(NC):
            w = W_l0b1[n, c] * (wmix[1][n, 0] if c == 0 else 1.0)  # [H, OUT]
            for hh in range(2):
                cc0 = OFF_WB1 + ((n * 2 + hh) * 3 + c) * 128
                WB1[:, cc0:cc0 + 128] = w[hh * 128:(hh + 1) * 128, :]
    for n in range(NE):
        for c in range(NC):
            w = W_l1b0[n, c] * (wmix[2][n, 0] if c == 0 else 1.0)  # [OUT, H]
            for hh in range(2):
                cc0 = OFF_W10 + (n * 6 + c * 2 + hh) * 128
                WB1[:, cc0:cc0 + 128] = w[:, hh * 128:(hh + 1) * 128]
    for n in range(NE):
        for c in range(NC):
            w = W_l1b1[n, c] * (wmix[3][n, 0] if c == 0 else 1.0)  # [H, OUT]
            for hh in range(2):
                cc0 = OFF_W11 + ((n * 2 + hh) * 3 + c) * 128
                WB1[:, cc0:cc0 + 128] = w[hh * 128:(hh + 1) * 128, :]

    # ---- exp bias [44,1]: rows 0:16 g0 (e*4+n), rows 32:44 g1 (d*4+e) ----
    gbias = np.zeros((44, 1), dtype=f32)
    for n in range(NE):
        for e in range(NE):
            gbias[e * 4 + n, 0] = inputs['bg0'][n, e] + inputs['beta0'][n, e]
    for d in range(ND):
        for e in range(NE):
            gbias[32 + d * 4 + e, 0] = inputs['bg1'][d, e] + inputs['beta1'][d, e]

    return {'WB0': _bf16(WB0), 'WB1': _bf16(WB1), 'gbias': gbias}, wmix


def prep_core_v2(inputs, r):
    lo, hi = r * B_LOC, (r + 1) * B_LOC
    xs = inputs['sparse_embs'][lo:hi].reshape(B_LOC, KSP).astype(np.float32)
    xT = np.zeros((128, 4 * B_LOC), dtype=ml_dtypes.bfloat16)
    xsT = _bf16(xs.T)                                     # [416, B_LOC]
    for kt in range(3):
        xT[:, kt * B_LOC:(kt + 1) * B_LOC] = xsT[kt * 128:(kt + 1) * 128]
    xT[0:32, 3 * B_LOC:4 * B_LOC] = xsT[384:416]
    xT[32:45, 3 * B_LOC:4 * B_LOC] = _bf16(inputs['dense_features'][lo:hi].astype(np.float32).T)
    # onehot12 rows d*4+e = [domain==d]
    dom = inputs['domain_ids'][lo:hi].astype(np.int64)
    oh12 = np.zeros((12, B_LOC), dtype=ml_dtypes.bfloat16)
    for d in range(ND):
        m = (dom == d).astype(np.float32)
        for e in range(NE):
            oh12[d * 4 + e] = _bf16(m)
    return {'xT': xT, 'oh12': oh12}


# ============================================================================
# program
# ============================================================================

def build_program_v2(wmix):
    nc = bacc.Bacc(trn_type="TRN2", target_bir_lowering=False, debug=False)

    t_xT = nc.dram_tensor('xT', [128, 4 * B_LOC], BF16, kind="ExternalInput").ap()
    t_oh = nc.dram_tensor('oh12', [12, B_LOC], BF16, kind="ExternalInput").ap()
    t_WB0 = nc.dram_tensor('WB0', [128, C_WB0], BF16, kind="ExternalInput").ap()
    t_WB1 = nc.dram_tensor('WB1', [128, C_WB1], BF16, kind="ExternalInput").ap()
    t_gb = nc.dram_tensor('gbias', [44, 1], F32, kind="ExternalInput").ap()
    t_out = nc.dram_tensor('out', [B_LOC, OUT], F32, kind="ExternalOutput").ap()

    uid = itertools.count()

    with tile.TileContext(nc) as tc:
        with (
            tc.tile_pool(name="wpool", bufs=1) as wpool,
            tc.tile_pool(name="xqpool", bufs=3) as xqpool,
            tc.tile_pool(name="gpool", bufs=2) as gpool,
            tc.tile_pool(name="apool", bufs=3) as apool,
            tc.tile_pool(name="hpool", bufs=2) as hpool,
            tc.tile_pool(name="bcpool", bufs=5) as bcpool,
            tc.tile_pool(name="opool", bufs=2) as opool,
            tc.tile_pool(name="ps_big", bufs=2, space="PSUM") as ps_big,
            tc.tile_pool(name="ps_b1", bufs=3, space="PSUM") as ps_b1,
            tc.tile_pool(name="ps_sm", bufs=1, space="PSUM") as ps_sm,
        ):
            # ---- prologue DMAs ----
            sWB0 = wpool.tile([128, C_WB0], BF16, tag="WB0", name="WB0")
            nc.sync.dma_start(sWB0[:], t_WB0)
            sGb = wpool.tile([44, 1], F32, tag="gbias", name="gbias")
            nc.sync.dma_start(sGb[:], t_gb)
            sXT = wpool.tile([128, 4 * B_LOC], BF16, tag="xT", name="xT")
            xr_d = t_xT.rearrange("p (k b) -> p k b", k=4)
            xr_s = sXT[:].rearrange("p (k b) -> p k b", k=4)
            nc.sync.dma_start(xr_s[:, :, 0:NBC], xr_d[:, :, 0:NBC])
            nc.sync.dma_start(xr_s[:, :, NBC:B_LOC], xr_d[:, :, NBC:B_LOC])
            # oh12 lands at partitions 32:44 so the phase4 TT against
            # e1 (gexp[32:44]) has matching base partitions
            sOh = wpool.tile([44, B_LOC], BF16, tag="oh12", name="oh12")
            nc.sync.dma_start(sOh[32:44, :], t_oh)
            sWB1 = wpool.tile([128, C_WB1], BF16, tag="WB1", name="WB1")
            nc.sync.dma_start(sWB1[:, OFF_WL0:OFF_WB1], t_WB1[:, OFF_WL0:OFF_WB1])
            nc.sync.dma_start(sWB1[:, OFF_WB1:C_WB1], t_WB1[:, OFF_WB1:C_WB1])

            def sg_l(kt):
                rows = 128 if kt < 3 else 64
                return sWB0[0:rows, OFF_SG + kt * 108: OFF_SG + (kt + 1) * 108]

            def gq_l(kt):
                rows = 128 if kt < 3 else 64
                return sWB0[0:rows, OFF_GQ + kt * 64: OFF_GQ + (kt + 1) * 64]

            sSel16 = sWB0[0:16, OFF_SEL16:OFF_SEL16 + 4]
            sSelBr = sWB0[0:4, OFF_SELBR:OFF_SELBR + 16]
            sS12R = sWB0[0:12, OFF_S12R:OFF_S12R + 36]
            sId = sWB0[:, OFF_ID:OFF_ID + 128]

            def bc16(k):
                return sWB0[0:16, OFF_BC16 + k * 128: OFF_BC16 + (k + 1) * 128]

            def bcd(e):
                return sWB0[0:4, OFF_BCD + e * 128: OFF_BCD + (e + 1) * 128]

            def wl0(kt, m):
                return sWB1[:, OFF_WL0 + kt * 3072 + m * 128: OFF_WL0 + kt * 3072 + (m + 1) * 128]

            def wb1(n, hh, c):
                o = OFF_WB1 + ((n * 2 + hh) * 3 + c) * 128
                return sWB1[:, o:o + 128]

            def w10(n, c, hh):
                o = OFF_W10 + (n * 6 + c * 2 + hh) * 128
                return sWB1[:, o:o + 128]

            def w11(n, hh, c):
                o = OFF_W11 + ((n * 2 + hh) * 3 + c) * 128
                return sWB1[:, o:o + 128]

            def xblk(kt, cc, rows=128):
                return sXT[0:rows, kt * B_LOC + cc: kt * B_LOC + cc + NBC]

            # per-chunk state
            e0n = [None] * NCHUNK
            e1bf = [None] * NCHUNK
            hAd = [None] * NCHUNK
            hB = [None] * NCHUNK
            mixed = [None] * NCHUNK
            hCd = [None] * NCHUNK
            h2 = [None] * NCHUNK

            # ============ P0: squares, gates, fm, softmax prep ============
            def phase0(ch):
                cc = ch * NBC
                xq = xqpool.tile([128, 4 * NBC], BF16, tag="xq", name=f"xq_{ch}")
                for kt in range(3):
                    nc.gpsimd.tensor_tensor(xq[:, kt * NBC:(kt + 1) * NBC],
                                            xblk(kt, cc), xblk(kt, cc), ALU.mult)
                nc.gpsimd.tensor_tensor(xq[0:64, 3 * NBC:4 * NBC],
                                        xblk(3, cc, 64), xblk(3, cc, 64), ALU.mult)

                sg_ps = ps_sm.tile([108, NBC], F32, tag="sm", name=f"sg_{ch}")
                for kt in range(4):
                    rows = 128 if kt < 3 else 64
                    nc.tensor.matmul(sg_ps[:], sg_l(kt), xblk(kt, cc, rows),
                                     start=(kt == 0), stop=(kt == 3))
                q_ps = ps_sm.tile([64, NBC], F32, tag="sm", name=f"q_{ch}")
                for kt in range(4):
                    rows = 128 if kt < 3 else 64
                    nc.tensor.matmul(q_ps[:], gq_l(kt),
                                     xq[0:rows, kt * NBC: kt * NBC + NBC],
                                     start=(kt == 0), stop=(kt == 3))
                ssq = gpool.tile([64, NBC], F32, tag="ssq", name=f"ssq_{ch}")
                nc.scalar.activation(ssq[:], sg_ps[0:64, :], AF.Square,
                                     scale=float(np.sqrt(0.5)))
                # gate exp (bias folded) -> bf16; rows 0:16 = e0, 32:44 = e1
                gexp = gpool.tile([44, NBC], BF16, tag="gexp", name=f"gexp_{ch}",
                                  bufs=NCHUNK)
                nc.scalar.activation(gexp[:], sg_ps[64:108, :], AF.Exp, bias=sGb[:, 0:1])
                e1bf[ch] = gexp[32:44, :]
                # fm rows -> xT kt3 block partitions 64:128
                nc.vector.tensor_tensor(sXT[64:128, 3 * B_LOC + cc: 3 * B_LOC + cc + NBC],
                                        ssq[:], q_ps[:], ALU.subtract)
                # expert-gate softmax: e0n = e0 * bcast16(1/sum)
                s_ps = ps_sm.tile([4, NBC], F32, tag="sm", name=f"s_{ch}")
                nc.tensor.matmul(s_ps[:], sSel16, gexp[0:16, :], start=True, stop=True)
                r0f = gpool.tile([4, NBC], F32, tag="r0f", name=f"r0f_{ch}")
                nc.vector.reciprocal_approx_fast(r0f[:], s_ps[:])
                r0 = gpool.tile([4, NBC], BF16, tag="r0", name=f"r0_{ch}")
                nc.scalar.copy(r0[:], r0f[:])
                rbc = ps_sm.tile([16, NBC], F32, tag="sm", name=f"rbc_{ch}")
                nc.tensor.matmul(rbc[:], sSelBr, r0[:], start=True, stop=True)
                t = gpool.tile([16, NBC], BF16, tag="e0n", name=f"e0n_{ch}",
                               bufs=NCHUNK)
                nc.vector.tensor_tensor(t[:], gexp[0:16, :], rbc[:], ALU.mult)
                e0n[ch] = t

            # ============ P1 pieces ============
            def l0b0_n(ch, n):
                cc = ch * NBC
                ht = hpool.tile([128, 2 * NBC], BF16, tag=f"hA{n}", name=f"hA{n}_{ch}")
                hAd[ch][n] = ht
                for c in range(NC):
                    p = ps_big.tile([128, 2 * NBC], F32, tag="big",
                                    name=f"pA{n}{c}_{ch}")
                    for hh in range(2):
                        m = n * 6 + c * 2 + hh
                        for kt in range(4):
                            nc.tensor.matmul(p[:, hh * NBC:(hh + 1) * NBC],
                                             wl0(kt, m), xblk(kt, cc),
                                             start=(kt == 0), stop=(kt == 3))
                    if c == 0:
                        nc.scalar.activation(ht[:], p[:], AF.Relu)
                    else:
                        fn = AF.Gelu_apprx_tanh if c == 1 else AF.Tanh
                        tmp = apool.tile([128, 2 * NBC], BF16, tag=f"td{c}",
                                         name=f"td{c}_{next(uid)}")
                        nc.scalar.activation(tmp[:], p[:], fn)
                        nc.vector.scalar_tensor_tensor(ht[:], tmp[:],
                                                       float(wmix[0][n, c]), ht[:],
                                                       ALU.mult, ALU.add)

            def l0b1_n(ch, n):
                ht = hpool.tile([128, NBC], BF16, tag=f"hB{n}", name=f"hB{n}_{ch}")
                hB[ch][n] = ht
                for c in range(NC):
                    p = ps_b1.tile([128, NBC], F32, tag="b1", name=f"pB{n}{c}_{ch}")
                    for hh in range(2):
                        nc.tensor.matmul(p[:], wb1(n, hh, c),
                                         hAd[ch][n][:, hh * NBC:(hh + 1) * NBC],
                                         start=(hh == 0), stop=(hh == 1))
                    if c == 0:
                        nc.scalar.activation(ht[:], p[:], AF.Relu)
                    else:
                        fn = AF.Gelu_apprx_tanh if c == 1 else AF.Tanh
                        tmp = apool.tile([128, NBC], BF16, tag=f"ts{c}",
                                         name=f"ts{c}_{next(uid)}")
                        nc.scalar.activation(tmp[:], p[:], fn)
                        nc.vector.scalar_tensor_tensor(ht[:], tmp[:],
                                                       float(wmix[1][n, c]), ht[:],
                                                       ALU.mult, ALU.add)

            # ============ P2: expert mixing for one n ============
            # multiplies on DVE (GpSimd cannot read PSUM), adds on Pool
            def phase2_n(ch, n):
                mul_eng = nc.vector
                add_eng = nc.gpsimd
                acc = bcpool.tile([128, NBC], BF16, tag=f"mix{n}", name=f"mix{n}_{ch}")
                mixed[ch][n] = acc
                for e in range(NE):
                    bp = ps_b1.tile([128, NBC], F32, tag="bp", name=f"bp{n}{e}_{ch}")
                    nc.tensor.matmul(bp[:], bc16(e * 4 + n), e0n[ch][:],
                                     start=True, stop=True)
                    if e == 0:
                        mul_eng.tensor_tensor(acc[:], hB[ch][0][:], bp[:], ALU.mult)
                    else:
                        t2 = bcpool.tile([128, NBC], BF16, tag="mixt",
                                         name=f"mixt{n}{e}_{ch}")
                        mul_eng.tensor_tensor(t2[:], hB[ch][e][:], bp[:], ALU.mult)
                        add_eng.tensor_tensor(acc[:], acc[:], t2[:], ALU.add)

            # ============ P3: L1b0 -> hCd ; L1b1 -> h2 ============
            def l1b0_n(ch, n):
                ht = hpool.tile([128, 2 * NBC], BF16, tag=f"hC{n}", name=f"hC{n}_{ch}")
                hCd[ch][n] = ht
                for c in range(NC):
                    p = ps_big.tile([128, 2 * NBC], F32, tag="big",
                                    name=f"pC{n}{c}_{ch}")
                    for hh in range(2):
                        nc.tensor.matmul(p[:, hh * NBC:(hh + 1) * NBC],
                                         w10(n, c, hh), mixed[ch][n][:],
                                         start=True, stop=True)
                    if c == 0:
                        nc.scalar.activation(ht[:], p[:], AF.Relu)
                    else:
                        fn = AF.Gelu_apprx_tanh if c == 1 else AF.Tanh
                        tmp = apool.tile([128, 2 * NBC], BF16, tag=f"td{c}",
                                         name=f"tc{c}_{next(uid)}")
                        nc.scalar.activation(tmp[:], p[:], fn)
                        nc.vector.scalar_tensor_tensor(ht[:], tmp[:],
                                                       float(wmix[2][n, c]), ht[:],
                                                       ALU.mult, ALU.add)

            def l1b1_n(ch, n):
                ht = hpool.tile([128, NBC], BF16, tag=f"h2{n}", name=f"h2{n}_{ch}")
                h2[ch][n] = ht
                for c in range(NC):
                    p = ps_b1.tile([128, NBC], F32, tag="b1", name=f"pD{n}{c}_{ch}")
                    for hh in range(2):
                        nc.tensor.matmul(p[:], w11(n, hh, c),
                                         hCd[ch][n][:, hh * NBC:(hh + 1) * NBC],
                                         start=(hh == 0), stop=(hh == 1))
                    if c == 0:
                        nc.vector.tensor_scalar(ht[:], p[:], 0.0, None, ALU.max)
                    else:
                        fn = AF.Gelu_apprx_tanh if c == 1 else AF.Tanh
                        tmp = apool.tile([128, NBC], BF16, tag=f"ts{c}",
                                         name=f"tu{c}_{next(uid)}")
                        nc.scalar.activation(tmp[:], p[:], fn)
                        nc.vector.scalar_tensor_tensor(ht[:], tmp[:],
                                                       float(wmix[3][n, c]), ht[:],
                                                       ALU.mult, ALU.add)

            # ============ P4: domain softmax-select + output ============
            def phase4(ch):
                cc = ch * NBC
                z = opool.tile([12, NBC], BF16, tag="z", name=f"z_{ch}", bufs=1)
                nc.gpsimd.tensor_tensor(z[:], e1bf[ch], sOh[32:44, cc:cc + NBC], ALU.mult)
                rs_ps = ps_sm.tile([36, NBC], F32, tag="sm", name=f"rs_{ch}")
                nc.tensor.matmul(rs_ps[:], sS12R, z[:], start=True, stop=True)
                rinv = opool.tile([1, NBC], F32, tag="rinv", name=f"rinv_{ch}", bufs=1)
                nc.vector.reciprocal_approx_fast(rinv[:], rs_ps[0:1, :])
                rb4 = opool.tile([4, NBC], F32, tag="rb4", name=f"rb4_{ch}", bufs=1)
                nc.gpsimd.partition_broadcast(rb4[:], rinv[:])
                dseln = opool.tile([4, NBC], BF16, tag="dseln", name=f"dseln_{ch}", bufs=1)
                nc.vector.tensor_tensor(dseln[:], rs_ps[32:36, :], rb4[:], ALU.mult)
                oacc = opool.tile([128, NBC], BF16, tag="oacc", name=f"oacc_{ch}", bufs=1)
                for e in range(NE):
                    bp = ps_b1.tile([128, NBC], F32, tag="bp", name=f"bpd{e}_{ch}")
                    nc.tensor.matmul(bp[:], bcd(e), dseln[:], start=True, stop=True)
                    if e == 0:
                        nc.vector.tensor_tensor(oacc[:], h2[ch][0][:], bp[:], ALU.mult)
                    else:
                        t2 = opool.tile([128, NBC], BF16, tag="omixt",
                                        name=f"ot{e}_{ch}", bufs=1)
                        nc.vector.tensor_tensor(t2[:], h2[ch][e][:], bp[:], ALU.mult)
                        nc.gpsimd.tensor_tensor(oacc[:], oacc[:], t2[:], ALU.add)
                tp = ps_sm.tile([128, NBC], BF16, tag="sm", name=f"tp_{ch}")
                for bt in range(4):
                    nc.tensor.transpose(tp[:, bt * 128:(bt + 1) * 128],
                                        oacc[:, bt * 128:(bt + 1) * 128], sId)
                oF = opool.tile([128, NBC], F32, tag="oF", name=f"oF_{ch}")
                nc.scalar.copy(oF[:], tp[:])
                for bt in range(4):
                    nc.sync.dma_start(t_out[cc + bt * 128: cc + (bt + 1) * 128, :],
                                      oF[:, bt * 128:(bt + 1) * 128])

            # ---- emission schedule: ph2(ch-1) interleaves into ph1(ch);
            # phase0 of later chunks spreads into earlier rounds; the
            # epilogue (last chunk's back half) chains per-n so it pipelines.
            phase0(0)
            phase0(1)
            for ch in range(NCHUNK):
                hAd[ch] = {}
                hB[ch] = {}
                if ch > 0:
                    mixed[ch - 1] = {}
                for n in range(NE):
                    l0b0_n(ch, n)
                    if ch > 0:
                        phase2_n(ch - 1, n)
                    if n == 1 and ch + 2 < NCHUNK:
                        phase0(ch + 2)
                for n in range(NE):
                    l0b1_n(ch, n)
                if ch > 0:
                    hCd[ch - 1] = {}
                    h2[ch - 1] = {}
                    for n in range(NE):
                        l1b0_n(ch - 1, n)
                    for n in range(NE):
                        l1b1_n(ch - 1, n)
                    phase4(ch - 1)
            ch = NCHUNK - 1
            mixed[ch] = {}
            hCd[ch] = {}
            h2[ch] = {}
            for n in range(NE):
                phase2_n(ch, n)
                l1b0_n(ch, n)
                l1b1_n(ch, n)
            phase4(ch)
    nc.compile()
    return nc


_CACHE = {}


def kernel(**inputs):
    zero_bias = all(np.abs(inputs[k]).max() == 0.0
                    for k in ('b_l0b0', 'b_l0b1', 'b_l1b0', 'b_l1b1'))
    if not zero_bias:
        raise NotImplementedError("v2 kernel requires zero expert biases")
    shared, wmix = prep_shared_v2(inputs)
    in_maps = []
    for r in range(N_CORES):
        m = dict(shared)
        m.update(prep_core_v2(inputs, r))
        in_maps.append(m)
    key = tuple(np.concatenate([w.ravel() for w in wmix]).tolist())
    if _CACHE.get('key') != key:
        _CACHE['nc'] = build_program_v2(wmix)
        _CACHE['key'] = key
    nc = _CACHE['nc']
    res = run_bass_kernel_spmd(nc, in_maps, core_ids=list(range(N_CORES)))
    out = np.concatenate([res.results[r]['out'] for r in range(N_CORES)], axis=0)
    return out.astype(np.float32)
